# revision 10
# baseline (speedup 1.0000x reference)
"""CBOW negative-sampling loss on 8 Trainium2 NeuronCores — v3.

v2.5 analysis: DVE was the wall (171us active = mult 5us + fold tree
5.7us per tile at the bf16 2x-mode rate), with DMA at 114us and
TensorE/ACT idle. GPSIMD cannot help (it shares DVE's second SBUF port
pair — exclusive lock, so DVE-2x and GPSIMD serialize). PE and ACT have
their own SBUF ports, so v3 moves the reduction onto them:

  - Host packs the gathered rows TRANSPOSED (d on partitions) in three
    d-chunks (0:128 / 128:256 / 256:300), j-major (j, p) in the free
    dim, plus the target row per chunk.
  - DVE does ONLY the broadcast multiply (3 ops/tile, 2x mode).
  - TensorE reduces over d (the partition dim) with an all-ones
    stationary column: per j-group of 4, three accumulating matmuls
    (one per chunk) write psum[g, (j', p)] = dots. 24 matmuls/tile.
  - ACT evacuates psum -> SBUF bf16; TensorE transposes [8, 128]
    blocks (identity matmul) to get [p, (j', g)]; ACT evacuates again.
  - Post-pass once per core on [128, 16*32]: sign multiply, stable
    softplus (Exp/Ln on ACT), weighted sum fused with accum_out.
  - loss = sum(per-core [128, 1]) / B on host.

Group-7 note: only 30 j-slots exist, so group 7 reuses columns for
j 26..29 (j 26, 27 duplicate group 6's tail); the duplicate slots get
weight 0 in the wz table.
"""

import sys

for _p in ("/opt/trn_rl_repo", "/opt/pypackages"):
    if _p not in sys.path:
        sys.path.append(_p)

import ml_dtypes
import numpy as np

import concourse.bass as bass
import concourse.bacc as bacc
import concourse.tile as tile
from concourse import mybir
from concourse.bass_utils import run_bass_kernel_spmd

V = 100000
D = 300
B = 16384
C = 10
K = 20
NCORES = 8
P = 128
NJ = C + K  # 30
R = NJ + 1
BCORE = B // NCORES  # 2048
NT = BCORE // P  # 16
D2 = D - 2 * P  # 44 rows in the third d-chunk
W1 = 2 * NJ * P + 2 * P  # 7936 (c0 + c1 + tgt0 + tgt1)
W2 = NJ * P + P  # 3968 (c2 + tgt2)
NG = 8  # j-groups of 4 -> 32 (j',g) slots per batch element
GW = 4 * P  # 512 columns per group

GNP = ml_dtypes.bfloat16
GDT = mybir.dt.bfloat16
_f32 = mybir.dt.float32


def _bc(sliced_ap, nj):
    """Broadcast a [P, w] slice across nj j-slots: [P, (0,nj), (1,w)]."""
    return bass.AP(
        sliced_ap.tensor,
        sliced_ap.offset,
        [sliced_ap.ap[0], [0, nj], sliced_ap.ap[-1]],
    )


def build_nc(nt: int):
    nc = bacc.Bacc(None, target_bir_lowering=False, debug=False)
    AF = mybir.ActivationFunctionType
    OP = mybir.AluOpType

    exp1 = nc.dram_tensor("exp1", [nt * P, W1], GDT, kind="ExternalInput")
    exp2 = nc.dram_tensor("exp2", [nt * D2, W2], GDT, kind="ExternalInput")
    stasd = nc.dram_tensor("stas", [P, NG * NG], GDT, kind="ExternalInput")
    identd = nc.dram_tensor("ident", [NG, NG], GDT, kind="ExternalInput")
    sgnd = nc.dram_tensor("sgn", [P, 4 * NG], GDT, kind="ExternalInput")
    wzd = nc.dram_tensor("wz", [P, 4 * NG], _f32, kind="ExternalInput")
    out = nc.dram_tensor("out", [P, 1], _f32, kind="ExternalOutput")

    with tile.TileContext(nc) as tc:
        with (
            tc.tile_pool(name="g1p", bufs=3) as g1p,
            tc.tile_pool(name="g2p", bufs=3) as g2p,
            tc.tile_pool(name="prp", bufs=2) as prp,
            tc.tile_pool(name="ysbp", bufs=2) as ysbp,
            tc.tile_pool(name="pyp", bufs=2, space="PSUM") as pyp,
            tc.tile_pool(name="ptp", bufs=2, space="PSUM") as ptp,
            tc.tile_pool(name="singles", bufs=1) as singles,
        ):
            stas = singles.tile([P, NG * NG], GDT)
            nc.sync.dma_start(out=stas[:], in_=stasd[:])
            ident = singles.tile([NG, NG], GDT)
            nc.sync.dma_start(out=ident[:], in_=identd[:])
            sgn = singles.tile([P, 4 * NG], GDT)
            nc.sync.dma_start(out=sgn[:], in_=sgnd[:])
            wz = singles.tile([P, 4 * NG], _f32)
            nc.sync.dma_start(out=wz[:], in_=wzd[:])

            Y = singles.tile([P, nt, 4 * NG], GDT)

            for t in range(nt):
                g1 = g1p.tile([P, W1], GDT, tag="g1")
                nc.sync.dma_start(out=g1[:], in_=exp1[t * P : (t + 1) * P])
                g2 = g2p.tile([D2, W2], GDT, tag="g2")
                nc.sync.dma_start(out=g2[:], in_=exp2[t * D2 : (t + 1) * D2])

                pr1 = prp.tile([P, 2 * NJ * P], GDT, tag="pr1")
                pr2 = prp.tile([D2, NJ * P], GDT, tag="pr2")
                nc.vector.tensor_tensor(
                    out=pr1[:, 0 : NJ * P],
                    in0=g1[:, 0 : NJ * P],
                    in1=_bc(g1[:, 2 * NJ * P : 2 * NJ * P + P], NJ),
                    op=OP.mult,
                )
                nc.vector.tensor_tensor(
                    out=pr1[:, NJ * P : 2 * NJ * P],
                    in0=g1[:, NJ * P : 2 * NJ * P],
                    in1=_bc(g1[:, 2 * NJ * P + P : 2 * NJ * P + 2 * P], NJ),
                    op=OP.mult,
                )
                nc.vector.tensor_tensor(
                    out=pr2[:],
                    in0=g2[:, 0 : NJ * P],
                    in1=_bc(g2[:, NJ * P : NJ * P + P], NJ),
                    op=OP.mult,
                )

                py = pyp.tile([NG, GW], _f32, tag="py")
                for g in range(NG):
                    # group 7 reuses the last 512 columns (j 26..29);
                    # stationary block g = indicator column -> psum row g
                    c0 = min(g * GW, NJ * P - GW)
                    sta = stas[:, g * NG : (g + 1) * NG]
                    nc.tensor.matmul(
                        py[:, :],
                        sta,
                        pr1[:, c0 : c0 + GW],
                        start=(g == 0),
                        stop=False,
                    )
                    nc.tensor.matmul(
                        py[:, :],
                        sta,
                        pr1[:, NJ * P + c0 : NJ * P + c0 + GW],
                        start=False,
                        stop=False,
                    )
                    nc.tensor.matmul(
                        py[:, :],
                        stas[0:D2, g * NG : (g + 1) * NG],
                        pr2[:, c0 : c0 + GW],
                        start=False,
                        stop=(g == NG - 1),
                    )

                ysb = ysbp.tile([NG, GW], GDT, tag="ysb")
                nc.scalar.copy(ysb[:], py[:])

                pt = ptp.tile([P, 4 * NG], GDT, tag="pt")
                for jp in range(4):
                    nc.tensor.transpose(
                        pt[:, jp * NG : (jp + 1) * NG],
                        ysb[:, jp * P : (jp + 1) * P],
                        ident[:],
                    )
                nc.scalar.copy(Y[:, t, :], pt[:])

            # ---- post-pass over all tiles: [P, nt*32] ----
            NW = nt * 4 * NG
            sg = singles.tile([P, nt, 4 * NG], GDT)
            z = singles.tile([P, nt, 4 * NG], GDT)
            nc.vector.tensor_tensor(
                out=z[:], in0=Y[:], in1=_bc(sgn[:], nt), op=OP.mult
            )
            rl = singles.tile([P, nt, 4 * NG], GDT)
            nc.vector.tensor_scalar_max(rl[:], z[:], 0.0)
            na = singles.tile([P, nt, 4 * NG], GDT)
            nc.vector.scalar_tensor_tensor(
                out=na[:],
                in0=z[:],
                scalar=-1.0,
                in1=z[:],
                op0=OP.mult,
                op1=OP.min,
            )
            e = singles.tile([P, nt, 4 * NG], _f32)
            nc.scalar.activation(e[:], na[:], AF.Exp)
            l = singles.tile([P, nt, 4 * NG], _f32)
            nc.scalar.activation(l[:], e[:], AF.Ln, bias=1.0)
            sp = singles.tile([P, nt, 4 * NG], _f32)
            nc.vector.tensor_tensor(out=sp[:], in0=rl[:], in1=l[:], op=OP.add)
            spw = singles.tile([P, nt, 4 * NG], _f32)
            acc = singles.tile([P, 1], _f32)
            nc.vector.scalar_tensor_tensor(
                out=spw[:],
                in0=sp[:],
                scalar=1.0,
                in1=_bc(wz[:], nt),
                op0=OP.mult,
                op1=OP.mult,
                accum_out=acc[:],
            )
            nc.sync.dma_start(out=out[:], in_=acc[:])

    nc.compile()
    return nc


_NC_CACHE: dict = {}


def _get_nc(nt: int):
    if nt not in _NC_CACHE:
        _NC_CACHE[nt] = build_nc(nt)
    return _NC_CACHE[nt]


def _qj():
    """j index for each of the 32 (j', g) slots; -1 = weight-0 duplicate."""
    j = np.zeros(4 * NG, dtype=np.int64)
    dup = np.zeros(4 * NG, dtype=bool)
    for jp in range(4):
        for g in range(NG):
            q = jp * NG + g
            if g < 7:
                j[q] = 4 * g + jp
            else:
                j[q] = 26 + jp
                dup[q] = jp < 2
    return j, dup


def kernel(i_emb, o_emb, context, target, neg_samples, _trace=False, _trace_kwargs=None):
    i_emb = np.asarray(i_emb, dtype=np.float32)
    o_emb = np.asarray(o_emb, dtype=np.float32)
    context = np.asarray(context).astype(np.int64)
    target = np.asarray(target).astype(np.int64)
    neg_samples = np.asarray(neg_samples).astype(np.int64)

    table = np.empty((2 * V, D), dtype=GNP)
    table[:V] = o_emb.astype(GNP)
    table[V:] = i_emb.astype(GNP)

    all_rows = np.concatenate(
        [context, neg_samples, target[:, None] + V], axis=1
    )
    expanded = table[all_rows]  # [B, R, D] bf16
    arr = expanded.reshape(NCORES, NT, P, R, D)
    aT = np.ascontiguousarray(arr.transpose(0, 1, 4, 3, 2))  # (c, t, d, j, p)

    exp1 = np.concatenate(
        [
            aT[:, :, 0:P, 0:NJ, :].reshape(NCORES, NT, P, NJ * P),
            aT[:, :, P : 2 * P, 0:NJ, :].reshape(NCORES, NT, P, NJ * P),
            aT[:, :, 0:P, NJ, :],
            aT[:, :, P : 2 * P, NJ, :],
        ],
        axis=3,
    )  # (8, 16, 128, 7936)
    exp2 = np.concatenate(
        [
            aT[:, :, 2 * P : D, 0:NJ, :].reshape(NCORES, NT, D2, NJ * P),
            aT[:, :, 2 * P : D, NJ, :],
        ],
        axis=3,
    )  # (8, 16, 44, 3968)

    jq, dup = _qj()
    sgn_row = np.where(jq < C, -1.0, 1.0).astype(GNP)
    wz_row = np.where(dup, 0.0, np.where(jq < C, 1.0 / C, 1.0)).astype(
        np.float32
    )
    stas = np.zeros((P, NG * NG), dtype=GNP)
    for g in range(NG):
        stas[:, g * NG + g] = 1.0
    consts = {
        "stas": stas,
        "ident": np.eye(NG, dtype=GNP),
        "sgn": np.tile(sgn_row, (P, 1)),
        "wz": np.tile(wz_row, (P, 1)),
    }

    nc = _get_nc(NT)

    in_maps = []
    for c in range(NCORES):
        in_maps.append(
            {
                "exp1": np.ascontiguousarray(exp1[c].reshape(NT * P, W1)),
                "exp2": np.ascontiguousarray(exp2[c].reshape(NT * D2, W2)),
                **consts,
            }
        )

    kw = {}
    if _trace:
        kw["trace"] = True
        if _trace_kwargs:
            kw.update(_trace_kwargs)
    res = run_bass_kernel_spmd(nc, in_maps, core_ids=list(range(NCORES)), **kw)

    total = np.float64(0.0)
    for c in range(NCORES):
        total += np.asarray(res.results[c]["out"], dtype=np.float64).sum()
    loss = np.float32(total / B)
    if _trace:
        return loss, res
    return loss


# revision 11
# speedup vs baseline: 1.2333x; 1.2333x over previous
"""CBOW negative-sampling loss on 8 Trainium2 NeuronCores — v3.

v2.5 analysis: DVE was the wall (171us active = mult 5us + fold tree
5.7us per tile at the bf16 2x-mode rate), with DMA at 114us and
TensorE/ACT idle. GPSIMD cannot help (it shares DVE's second SBUF port
pair — exclusive lock, so DVE-2x and GPSIMD serialize). PE and ACT have
their own SBUF ports, so v3 moves the reduction onto them:

  - Host packs the gathered rows TRANSPOSED (d on partitions) in three
    d-chunks (0:128 / 128:256 / 256:300), j-major (j, p) in the free
    dim, plus the target row per chunk.
  - DVE does ONLY the broadcast multiply (3 ops/tile, 2x mode).
  - TensorE reduces over d (the partition dim) with an all-ones
    stationary column: per j-group of 4, three accumulating matmuls
    (one per chunk) write psum[g, (j', p)] = dots. 24 matmuls/tile.
  - ACT evacuates psum -> SBUF bf16; TensorE transposes [8, 128]
    blocks (identity matmul) to get [p, (j', g)]; ACT evacuates again.
  - Post-pass once per core on [128, 16*32]: sign multiply, stable
    softplus (Exp/Ln on ACT), weighted sum fused with accum_out.
  - loss = sum(per-core [128, 1]) / B on host.

Group-7 note: only 30 j-slots exist, so group 7 reuses columns for
j 26..29 (j 26, 27 duplicate group 6's tail); the duplicate slots get
weight 0 in the wz table.
"""

import sys

for _p in ("/opt/trn_rl_repo", "/opt/pypackages"):
    if _p not in sys.path:
        sys.path.append(_p)

import ml_dtypes
import numpy as np

import concourse.bass as bass
import concourse.bacc as bacc
import concourse.tile as tile
from concourse import mybir
from concourse.bass_utils import run_bass_kernel_spmd

V = 100000
D = 300
B = 16384
C = 10
K = 20
NCORES = 8
P = 128
NJ = C + K  # 30
R = NJ + 1
BCORE = B // NCORES  # 2048
NT = BCORE // P  # 16
D2 = D - 2 * P  # 44 rows in the third d-chunk
W1 = 2 * NJ * P + 2 * P  # 7936 (c0 + c1 + tgt0 + tgt1)
W2 = NJ * P + P  # 3968 (c2 + tgt2)
NG = 8  # j-groups of 4 -> 32 (j',g) slots per batch element
GW = 4 * P  # 512 columns per group

GNP = ml_dtypes.bfloat16
GDT = mybir.dt.bfloat16
_f32 = mybir.dt.float32


def _bc(sliced_ap, nj):
    """Broadcast a [P, w] slice across nj j-slots: [P, (0,nj), (1,w)]."""
    return bass.AP(
        sliced_ap.tensor,
        sliced_ap.offset,
        [sliced_ap.ap[0], [0, nj], sliced_ap.ap[-1]],
    )


def build_nc(nt: int):
    nc = bacc.Bacc(None, target_bir_lowering=False, debug=False)
    AF = mybir.ActivationFunctionType
    OP = mybir.AluOpType

    exp1 = nc.dram_tensor("exp1", [nt * P, W1], GDT, kind="ExternalInput")
    exp2 = nc.dram_tensor("exp2", [nt * D2, W2], GDT, kind="ExternalInput")
    stasd = nc.dram_tensor("stas", [P, NG * NG], GDT, kind="ExternalInput")
    identd = nc.dram_tensor("ident", [NG, NG], GDT, kind="ExternalInput")
    sgnd = nc.dram_tensor("sgn", [P, 4 * NG], GDT, kind="ExternalInput")
    wzd = nc.dram_tensor("wz", [P, 4 * NG], _f32, kind="ExternalInput")
    out = nc.dram_tensor("out", [P, 1], _f32, kind="ExternalOutput")

    with tile.TileContext(nc) as tc:
        with (
            tc.tile_pool(name="g1p", bufs=3) as g1p,
            tc.tile_pool(name="g2p", bufs=3) as g2p,
            tc.tile_pool(name="prp", bufs=2) as prp,
            tc.tile_pool(name="ysbp", bufs=2) as ysbp,
            tc.tile_pool(name="pyp", bufs=2, space="PSUM") as pyp,
            tc.tile_pool(name="ptp", bufs=2, space="PSUM") as ptp,
            tc.tile_pool(name="singles", bufs=1) as singles,
        ):
            stas = singles.tile([P, NG * NG], GDT)
            nc.sync.dma_start(out=stas[:], in_=stasd[:])
            ident = singles.tile([NG, NG], GDT)
            nc.sync.dma_start(out=ident[:], in_=identd[:])
            sgn = singles.tile([P, 4 * NG], GDT)
            nc.sync.dma_start(out=sgn[:], in_=sgnd[:])
            wz = singles.tile([P, 4 * NG], _f32)
            nc.sync.dma_start(out=wz[:], in_=wzd[:])

            Y = singles.tile([P, nt, 4 * NG], GDT)

            for t in range(nt):
                g1 = g1p.tile([P, W1], GDT, tag="g1")
                nc.sync.dma_start(out=g1[:], in_=exp1[t * P : (t + 1) * P])
                g2 = g2p.tile([D2, W2], GDT, tag="g2")
                nc.sync.dma_start(out=g2[:], in_=exp2[t * D2 : (t + 1) * D2])

                pr1 = prp.tile([P, 2 * NJ * P], GDT, tag="pr1")
                pr2 = prp.tile([D2, NJ * P], GDT, tag="pr2")
                nc.vector.tensor_tensor(
                    out=pr1[:, 0 : NJ * P],
                    in0=g1[:, 0 : NJ * P],
                    in1=_bc(g1[:, 2 * NJ * P : 2 * NJ * P + P], NJ),
                    op=OP.mult,
                )
                nc.vector.tensor_tensor(
                    out=pr1[:, NJ * P : 2 * NJ * P],
                    in0=g1[:, NJ * P : 2 * NJ * P],
                    in1=_bc(g1[:, 2 * NJ * P + P : 2 * NJ * P + 2 * P], NJ),
                    op=OP.mult,
                )
                nc.vector.tensor_tensor(
                    out=pr2[:],
                    in0=g2[:, 0 : NJ * P],
                    in1=_bc(g2[:, NJ * P : NJ * P + P], NJ),
                    op=OP.mult,
                )

                # pre-sum chunks c0+c1 on DVE (one 2x add) so TensorE
                # only streams 2 passes per group instead of 3
                ps = prp.tile([P, NJ * P], GDT, tag="ps")
                nc.vector.tensor_add(
                    out=ps[:],
                    in0=pr1[:, 0 : NJ * P],
                    in1=pr1[:, NJ * P : 2 * NJ * P],
                )

                py = pyp.tile([NG, GW], _f32, tag="py")
                for g in range(NG):
                    # group 7 reuses the last 512 columns (j 26..29);
                    # stationary block g = indicator column -> psum row g
                    c0 = min(g * GW, NJ * P - GW)
                    sta = stas[:, g * NG : (g + 1) * NG]
                    nc.tensor.matmul(
                        py[:, :],
                        sta,
                        ps[:, c0 : c0 + GW],
                        start=(g == 0),
                        stop=False,
                    )
                    nc.tensor.matmul(
                        py[:, :],
                        stas[0:D2, g * NG : (g + 1) * NG],
                        pr2[:, c0 : c0 + GW],
                        start=False,
                        stop=(g == NG - 1),
                    )

                ysb = ysbp.tile([NG, GW], GDT, tag="ysb")
                nc.scalar.copy(ysb[:], py[:])

                pt = ptp.tile([P, 4 * NG], GDT, tag="pt")
                for jp in range(4):
                    nc.tensor.transpose(
                        pt[:, jp * NG : (jp + 1) * NG],
                        ysb[:, jp * P : (jp + 1) * P],
                        ident[:],
                    )
                nc.scalar.copy(Y[:, t, :], pt[:])

            # ---- post-pass over all tiles: [P, nt*32] ----
            NW = nt * 4 * NG
            sg = singles.tile([P, nt, 4 * NG], GDT)
            z = singles.tile([P, nt, 4 * NG], GDT)
            nc.vector.tensor_tensor(
                out=z[:], in0=Y[:], in1=_bc(sgn[:], nt), op=OP.mult
            )
            rl = singles.tile([P, nt, 4 * NG], GDT)
            nc.vector.tensor_scalar_max(rl[:], z[:], 0.0)
            na = singles.tile([P, nt, 4 * NG], GDT)
            nc.vector.scalar_tensor_tensor(
                out=na[:],
                in0=z[:],
                scalar=-1.0,
                in1=z[:],
                op0=OP.mult,
                op1=OP.min,
            )
            e = singles.tile([P, nt, 4 * NG], _f32)
            nc.scalar.activation(e[:], na[:], AF.Exp)
            l = singles.tile([P, nt, 4 * NG], _f32)
            nc.scalar.activation(l[:], e[:], AF.Ln, bias=1.0)
            sp = singles.tile([P, nt, 4 * NG], _f32)
            nc.vector.tensor_tensor(out=sp[:], in0=rl[:], in1=l[:], op=OP.add)
            spw = singles.tile([P, nt, 4 * NG], _f32)
            acc = singles.tile([P, 1], _f32)
            nc.vector.scalar_tensor_tensor(
                out=spw[:],
                in0=sp[:],
                scalar=1.0,
                in1=_bc(wz[:], nt),
                op0=OP.mult,
                op1=OP.mult,
                accum_out=acc[:],
            )
            nc.sync.dma_start(out=out[:], in_=acc[:])

    nc.compile()
    return nc


_NC_CACHE: dict = {}


def _get_nc(nt: int):
    if nt not in _NC_CACHE:
        _NC_CACHE[nt] = build_nc(nt)
    return _NC_CACHE[nt]


def _qj():
    """j index for each of the 32 (j', g) slots; -1 = weight-0 duplicate."""
    j = np.zeros(4 * NG, dtype=np.int64)
    dup = np.zeros(4 * NG, dtype=bool)
    for jp in range(4):
        for g in range(NG):
            q = jp * NG + g
            if g < 7:
                j[q] = 4 * g + jp
            else:
                j[q] = 26 + jp
                dup[q] = jp < 2
    return j, dup


def kernel(i_emb, o_emb, context, target, neg_samples, _trace=False, _trace_kwargs=None):
    i_emb = np.asarray(i_emb, dtype=np.float32)
    o_emb = np.asarray(o_emb, dtype=np.float32)
    context = np.asarray(context).astype(np.int64)
    target = np.asarray(target).astype(np.int64)
    neg_samples = np.asarray(neg_samples).astype(np.int64)

    table = np.empty((2 * V, D), dtype=GNP)
    table[:V] = o_emb.astype(GNP)
    table[V:] = i_emb.astype(GNP)

    all_rows = np.concatenate(
        [context, neg_samples, target[:, None] + V], axis=1
    )
    expanded = table[all_rows]  # [B, R, D] bf16
    arr = expanded.reshape(NCORES, NT, P, R, D)
    aT = np.ascontiguousarray(arr.transpose(0, 1, 4, 3, 2))  # (c, t, d, j, p)

    exp1 = np.concatenate(
        [
            aT[:, :, 0:P, 0:NJ, :].reshape(NCORES, NT, P, NJ * P),
            aT[:, :, P : 2 * P, 0:NJ, :].reshape(NCORES, NT, P, NJ * P),
            aT[:, :, 0:P, NJ, :],
            aT[:, :, P : 2 * P, NJ, :],
        ],
        axis=3,
    )  # (8, 16, 128, 7936)
    exp2 = np.concatenate(
        [
            aT[:, :, 2 * P : D, 0:NJ, :].reshape(NCORES, NT, D2, NJ * P),
            aT[:, :, 2 * P : D, NJ, :],
        ],
        axis=3,
    )  # (8, 16, 44, 3968)

    jq, dup = _qj()
    sgn_row = np.where(jq < C, -1.0, 1.0).astype(GNP)
    wz_row = np.where(dup, 0.0, np.where(jq < C, 1.0 / C, 1.0)).astype(
        np.float32
    )
    stas = np.zeros((P, NG * NG), dtype=GNP)
    for g in range(NG):
        stas[:, g * NG + g] = 1.0
    consts = {
        "stas": stas,
        "ident": np.eye(NG, dtype=GNP),
        "sgn": np.tile(sgn_row, (P, 1)),
        "wz": np.tile(wz_row, (P, 1)),
    }

    nc = _get_nc(NT)

    in_maps = []
    for c in range(NCORES):
        in_maps.append(
            {
                "exp1": np.ascontiguousarray(exp1[c].reshape(NT * P, W1)),
                "exp2": np.ascontiguousarray(exp2[c].reshape(NT * D2, W2)),
                **consts,
            }
        )

    kw = {}
    if _trace:
        kw["trace"] = True
        if _trace_kwargs:
            kw.update(_trace_kwargs)
    res = run_bass_kernel_spmd(nc, in_maps, core_ids=list(range(NCORES)), **kw)

    total = np.float64(0.0)
    for c in range(NCORES):
        total += np.asarray(res.results[c]["out"], dtype=np.float64).sum()
    loss = np.float32(total / B)
    if _trace:
        return loss, res
    return loss


# revision 12
# speedup vs baseline: 1.2710x; 1.0305x over previous
"""CBOW negative-sampling loss on 8 Trainium2 NeuronCores — v3.

v2.5 analysis: DVE was the wall (171us active = mult 5us + fold tree
5.7us per tile at the bf16 2x-mode rate), with DMA at 114us and
TensorE/ACT idle. GPSIMD cannot help (it shares DVE's second SBUF port
pair — exclusive lock, so DVE-2x and GPSIMD serialize). PE and ACT have
their own SBUF ports, so v3 moves the reduction onto them:

  - Host packs the gathered rows TRANSPOSED (d on partitions) in three
    d-chunks (0:128 / 128:256 / 256:300), j-major (j, p) in the free
    dim, plus the target row per chunk.
  - DVE does ONLY the broadcast multiply (3 ops/tile, 2x mode).
  - TensorE reduces over d (the partition dim) with an all-ones
    stationary column: per j-group of 4, three accumulating matmuls
    (one per chunk) write psum[g, (j', p)] = dots. 24 matmuls/tile.
  - ACT evacuates psum -> SBUF bf16; TensorE transposes [8, 128]
    blocks (identity matmul) to get [p, (j', g)]; ACT evacuates again.
  - Post-pass once per core on [128, 16*32]: sign multiply, stable
    softplus (Exp/Ln on ACT), weighted sum fused with accum_out.
  - loss = sum(per-core [128, 1]) / B on host.

Group-7 note: only 30 j-slots exist, so group 7 reuses columns for
j 26..29 (j 26, 27 duplicate group 6's tail); the duplicate slots get
weight 0 in the wz table.
"""

import sys

for _p in ("/opt/trn_rl_repo", "/opt/pypackages"):
    if _p not in sys.path:
        sys.path.append(_p)

import ml_dtypes
import numpy as np

import concourse.bass as bass
import concourse.bacc as bacc
import concourse.tile as tile
from concourse import mybir
from concourse.bass_utils import run_bass_kernel_spmd

V = 100000
D = 300
B = 16384
C = 10
K = 20
NCORES = 8
P = 128
NJ = C + K  # 30
R = NJ + 1
BCORE = B // NCORES  # 2048
NT = BCORE // P  # 16
D2 = D - 2 * P  # 44 rows in the third d-chunk
W1 = 2 * NJ * P + 2 * P  # 7936 (c0 + c1 + tgt0 + tgt1)
W2 = NJ * P + P  # 3968 (c2 + tgt2)
NG = 8  # j-groups of 4 -> 32 (j',g) slots per batch element
GW = 4 * P  # 512 columns per group

GNP = ml_dtypes.bfloat16
GDT = mybir.dt.bfloat16
_f32 = mybir.dt.float32


def _bc(sliced_ap, nj):
    """Broadcast a [P, w] slice across nj j-slots: [P, (0,nj), (1,w)]."""
    return bass.AP(
        sliced_ap.tensor,
        sliced_ap.offset,
        [sliced_ap.ap[0], [0, nj], sliced_ap.ap[-1]],
    )


def build_nc(nt: int):
    nc = bacc.Bacc(None, target_bir_lowering=False, debug=False)
    AF = mybir.ActivationFunctionType
    OP = mybir.AluOpType

    exp1 = nc.dram_tensor("exp1", [nt * P, W1], GDT, kind="ExternalInput")
    exp2 = nc.dram_tensor("exp2", [nt * D2, W2], GDT, kind="ExternalInput")
    stasd = nc.dram_tensor("stas", [P, NG * NG], GDT, kind="ExternalInput")
    identd = nc.dram_tensor("ident", [NG, NG], GDT, kind="ExternalInput")
    sgnd = nc.dram_tensor("sgn", [P, 4 * NG], GDT, kind="ExternalInput")
    wzd = nc.dram_tensor("wz", [P, 4 * NG], _f32, kind="ExternalInput")
    out = nc.dram_tensor("out", [P, 1], _f32, kind="ExternalOutput")

    with tile.TileContext(nc) as tc:
        with (
            tc.tile_pool(name="g1p", bufs=3) as g1p,
            tc.tile_pool(name="g2p", bufs=3) as g2p,
            tc.tile_pool(name="prp", bufs=2) as prp,
            tc.tile_pool(name="ysbp", bufs=2) as ysbp,
            tc.tile_pool(name="pyp", bufs=2, space="PSUM") as pyp,
            tc.tile_pool(name="ptp", bufs=2, space="PSUM") as ptp,
            tc.tile_pool(name="singles", bufs=1) as singles,
        ):
            stas = singles.tile([P, NG * NG], GDT)
            nc.sync.dma_start(out=stas[:], in_=stasd[:])
            ident = singles.tile([NG, NG], GDT)
            nc.sync.dma_start(out=ident[:], in_=identd[:])
            sgn = singles.tile([P, 4 * NG], GDT)
            nc.sync.dma_start(out=sgn[:], in_=sgnd[:])
            wz = singles.tile([P, 4 * NG], _f32)
            nc.sync.dma_start(out=wz[:], in_=wzd[:])

            Y = singles.tile([P, nt, 4 * NG], GDT)

            for t in range(nt):
                g1 = g1p.tile([P, W1], GDT, tag="g1")
                nc.sync.dma_start(out=g1[:], in_=exp1[t * P : (t + 1) * P])
                g2 = g2p.tile([D2, W2], GDT, tag="g2")
                nc.sync.dma_start(out=g2[:], in_=exp2[t * D2 : (t + 1) * D2])

                pr1 = prp.tile([P, 2 * NJ * P], GDT, tag="pr1")
                pr2 = prp.tile([D2, NJ * P], GDT, tag="pr2")
                nc.vector.tensor_tensor(
                    out=pr1[:, 0 : NJ * P],
                    in0=g1[:, 0 : NJ * P],
                    in1=_bc(g1[:, 2 * NJ * P : 2 * NJ * P + P], NJ),
                    op=OP.mult,
                )
                nc.vector.tensor_tensor(
                    out=pr1[:, NJ * P : 2 * NJ * P],
                    in0=g1[:, NJ * P : 2 * NJ * P],
                    in1=_bc(g1[:, 2 * NJ * P + P : 2 * NJ * P + 2 * P], NJ),
                    op=OP.mult,
                )
                nc.vector.tensor_tensor(
                    out=pr2[:],
                    in0=g2[:, 0 : NJ * P],
                    in1=_bc(g2[:, NJ * P : NJ * P + P], NJ),
                    op=OP.mult,
                )

                # pre-sum chunks c0+c1 on DVE (one 2x add) so TensorE
                # only streams 2 passes per group instead of 3
                ps = prp.tile([P, NJ * P], GDT, tag="ps")
                nc.vector.tensor_add(
                    out=ps[:],
                    in0=pr1[:, 0 : NJ * P],
                    in1=pr1[:, NJ * P : 2 * NJ * P],
                )
                # fold chunk 2 of the first NPRE groups into ps on DVE
                # (in-place, rows 0:44) to take matmuls off TensorE
                NPRE = 2
                nc.vector.tensor_add(
                    out=ps[0:D2, 0 : NPRE * GW],
                    in0=pr2[:, 0 : NPRE * GW],
                    in1=ps[0:D2, 0 : NPRE * GW],
                )

                py = pyp.tile([NG, GW], _f32, tag="py")
                for g in range(NG):
                    # group 7 reuses the last 512 columns (j 26..29);
                    # stationary block g = indicator column -> psum row g
                    c0 = min(g * GW, NJ * P - GW)
                    sta = stas[:, g * NG : (g + 1) * NG]
                    nc.tensor.matmul(
                        py[:, :],
                        sta,
                        ps[:, c0 : c0 + GW],
                        start=(g == 0),
                        stop=False,
                    )
                    if g >= NPRE:
                        nc.tensor.matmul(
                            py[:, :],
                            stas[0:D2, g * NG : (g + 1) * NG],
                            pr2[:, c0 : c0 + GW],
                            start=False,
                            stop=(g == NG - 1),
                        )

                ysb = ysbp.tile([NG, GW], GDT, tag="ysb")
                nc.scalar.copy(ysb[:], py[:])

                pt = ptp.tile([P, 4 * NG], GDT, tag="pt")
                for jp in range(4):
                    nc.tensor.transpose(
                        pt[:, jp * NG : (jp + 1) * NG],
                        ysb[:, jp * P : (jp + 1) * P],
                        ident[:],
                    )
                nc.scalar.copy(Y[:, t, :], pt[:])

            # ---- post-pass over all tiles: [P, nt*32] ----
            NW = nt * 4 * NG
            sg = singles.tile([P, nt, 4 * NG], GDT)
            z = singles.tile([P, nt, 4 * NG], GDT)
            nc.vector.tensor_tensor(
                out=z[:], in0=Y[:], in1=_bc(sgn[:], nt), op=OP.mult
            )
            rl = singles.tile([P, nt, 4 * NG], GDT)
            nc.vector.tensor_scalar_max(rl[:], z[:], 0.0)
            na = singles.tile([P, nt, 4 * NG], GDT)
            nc.vector.scalar_tensor_tensor(
                out=na[:],
                in0=z[:],
                scalar=-1.0,
                in1=z[:],
                op0=OP.mult,
                op1=OP.min,
            )
            e = singles.tile([P, nt, 4 * NG], _f32)
            nc.scalar.activation(e[:], na[:], AF.Exp)
            l = singles.tile([P, nt, 4 * NG], _f32)
            nc.scalar.activation(l[:], e[:], AF.Ln, bias=1.0)
            sp = singles.tile([P, nt, 4 * NG], _f32)
            nc.vector.tensor_tensor(out=sp[:], in0=rl[:], in1=l[:], op=OP.add)
            spw = singles.tile([P, nt, 4 * NG], _f32)
            acc = singles.tile([P, 1], _f32)
            nc.vector.scalar_tensor_tensor(
                out=spw[:],
                in0=sp[:],
                scalar=1.0,
                in1=_bc(wz[:], nt),
                op0=OP.mult,
                op1=OP.mult,
                accum_out=acc[:],
            )
            nc.sync.dma_start(out=out[:], in_=acc[:])

    nc.compile()
    return nc


_NC_CACHE: dict = {}


def _get_nc(nt: int):
    if nt not in _NC_CACHE:
        _NC_CACHE[nt] = build_nc(nt)
    return _NC_CACHE[nt]


def _qj():
    """j index for each of the 32 (j', g) slots; -1 = weight-0 duplicate."""
    j = np.zeros(4 * NG, dtype=np.int64)
    dup = np.zeros(4 * NG, dtype=bool)
    for jp in range(4):
        for g in range(NG):
            q = jp * NG + g
            if g < 7:
                j[q] = 4 * g + jp
            else:
                j[q] = 26 + jp
                dup[q] = jp < 2
    return j, dup


def kernel(i_emb, o_emb, context, target, neg_samples, _trace=False, _trace_kwargs=None):
    i_emb = np.asarray(i_emb, dtype=np.float32)
    o_emb = np.asarray(o_emb, dtype=np.float32)
    context = np.asarray(context).astype(np.int64)
    target = np.asarray(target).astype(np.int64)
    neg_samples = np.asarray(neg_samples).astype(np.int64)

    table = np.empty((2 * V, D), dtype=GNP)
    table[:V] = o_emb.astype(GNP)
    table[V:] = i_emb.astype(GNP)

    all_rows = np.concatenate(
        [context, neg_samples, target[:, None] + V], axis=1
    )
    expanded = table[all_rows]  # [B, R, D] bf16
    arr = expanded.reshape(NCORES, NT, P, R, D)
    aT = np.ascontiguousarray(arr.transpose(0, 1, 4, 3, 2))  # (c, t, d, j, p)

    exp1 = np.concatenate(
        [
            aT[:, :, 0:P, 0:NJ, :].reshape(NCORES, NT, P, NJ * P),
            aT[:, :, P : 2 * P, 0:NJ, :].reshape(NCORES, NT, P, NJ * P),
            aT[:, :, 0:P, NJ, :],
            aT[:, :, P : 2 * P, NJ, :],
        ],
        axis=3,
    )  # (8, 16, 128, 7936)
    exp2 = np.concatenate(
        [
            aT[:, :, 2 * P : D, 0:NJ, :].reshape(NCORES, NT, D2, NJ * P),
            aT[:, :, 2 * P : D, NJ, :],
        ],
        axis=3,
    )  # (8, 16, 44, 3968)

    jq, dup = _qj()
    sgn_row = np.where(jq < C, -1.0, 1.0).astype(GNP)
    wz_row = np.where(dup, 0.0, np.where(jq < C, 1.0 / C, 1.0)).astype(
        np.float32
    )
    stas = np.zeros((P, NG * NG), dtype=GNP)
    for g in range(NG):
        stas[:, g * NG + g] = 1.0
    consts = {
        "stas": stas,
        "ident": np.eye(NG, dtype=GNP),
        "sgn": np.tile(sgn_row, (P, 1)),
        "wz": np.tile(wz_row, (P, 1)),
    }

    nc = _get_nc(NT)

    in_maps = []
    for c in range(NCORES):
        in_maps.append(
            {
                "exp1": np.ascontiguousarray(exp1[c].reshape(NT * P, W1)),
                "exp2": np.ascontiguousarray(exp2[c].reshape(NT * D2, W2)),
                **consts,
            }
        )

    kw = {}
    if _trace:
        kw["trace"] = True
        if _trace_kwargs:
            kw.update(_trace_kwargs)
    res = run_bass_kernel_spmd(nc, in_maps, core_ids=list(range(NCORES)), **kw)

    total = np.float64(0.0)
    for c in range(NCORES):
        total += np.asarray(res.results[c]["out"], dtype=np.float64).sum()
    loss = np.float32(total / B)
    if _trace:
        return loss, res
    return loss
